# revision 5
# baseline (speedup 1.0000x reference)
"""MaxSim contrastive loss kernel for 8 Trainium2 NeuronCores.

Data-parallel over the batch dim (16 examples per core). Per example the
device computes pos/neg logits (pre-temperature, split in two query-tile
halves); the host finishes with the 2-class cross-entropy and batch mean.
"""
import numpy as np

B, LQ, LK, D = 128, 256, 512, 1024
NCORES = 8
EX = B // NCORES  # examples per core
TEMP = 0.05

P = 128
QT_TILES = LQ // P   # 2 query tiles
KT_TILES = LK // P   # 4 key tiles
DC = D // P          # 8 contraction chunks


def _split_multiwait_bir(bir_bytes):
    """This container's walrus build rejects >1 sync-wait per instruction.
    Split any multi-wait instruction into single-wait NoOps + the original."""
    import json

    j = json.loads(bir_bytes)
    ctr = 0
    changed = False
    for f in j.get("functions", []):
        for bb in f.get("blocks", []):
            out = []
            for ins in bb.get("instructions", []):
                si = ins.get("sync_info")
                if si:
                    waits = si.get("on_wait") or []
                    if len(waits) > 1:
                        changed = True
                        for w in waits[:-1]:
                            nop = {
                                "engine": ins["engine"],
                                "ins": [],
                                "outs": [],
                                "name": f"I-wsplit-{ctr}",
                                "opcode": "NoOp",
                                "sync_info": {"on_update": [], "on_wait": [w]},
                            }
                            if "debug" in ins:
                                nop["debug"] = ins["debug"]
                            out.append(nop)
                            ctr += 1
                        si["on_wait"] = [waits[-1]]
                out.append(ins)
            bb["instructions"] = out
    if not changed:
        return bir_bytes
    return json.dumps(j).encode()


def _install_patches():
    import concourse.bass_utils as _bu
    import concourse.bass2jax as _b2j

    if getattr(_bu, "_multiwait_patched", False):
        return
    orig = _bu.compile_bir_kernel

    def patched(bir_json, tmpdir, neff_name="file.neff"):
        if isinstance(bir_json, str):
            bir_json = bir_json.encode()
        return orig(_split_multiwait_bir(bir_json), tmpdir, neff_name)

    _bu.compile_bir_kernel = patched
    _b2j.compile_bir_kernel = patched
    _bu._multiwait_patched = True


def _build(n_ex=EX, use_f32r=False):
    import concourse.bass as bass
    import concourse.tile as tile
    from concourse import masks, mybir

    f32 = mybir.dt.float32
    i32 = mybir.dt.int32
    X = mybir.AxisListType.X
    MULT = mybir.AluOpType.mult
    Sq = mybir.ActivationFunctionType.Square

    mm_dt = mybir.dt.float32r if use_f32r else f32

    nc = bass.Bass()
    q_d = nc.dram_tensor("q", [n_ex, LQ, D], f32, kind="ExternalInput")
    pk_d = nc.dram_tensor("pk", [n_ex, LK, D], f32, kind="ExternalInput")
    nk_d = nc.dram_tensor("nk", [n_ex, LK, D], f32, kind="ExternalInput")
    qm_d = nc.dram_tensor("qm", [n_ex, LQ], i32, kind="ExternalInput")
    pm_d = nc.dram_tensor("pm", [n_ex, LK], i32, kind="ExternalInput")
    nm_d = nc.dram_tensor("nm", [n_ex, LK], i32, kind="ExternalInput")
    out_d = nc.dram_tensor("out", [n_ex * 4, 1], f32, kind="ExternalOutput")

    with tile.TileContext(nc) as tc:
        with (
            tc.tile_pool(name="const", bufs=1) as constp,
            tc.tile_pool(name="mask", bufs=1) as maskp,
            tc.tile_pool(name="qnat", bufs=2) as qnatp,
            tc.tile_pool(name="knat", bufs=3) as knatp,
            tc.tile_pool(name="qT", bufs=2) as qTp,
            tc.tile_pool(name="kT", bufs=3) as kTp,
            tc.tile_pool(name="scratch", bufs=2) as scrp,
            tc.tile_pool(name="temps", bufs=6) as tmpp,
            tc.tile_pool(name="psum_kT", bufs=2, space="PSUM") as ps_kT,
            tc.tile_pool(name="psum_qT", bufs=2, space="PSUM") as ps_qT,
            tc.tile_pool(name="psum_S", bufs=2, space="PSUM") as ps_S,
            tc.tile_pool(name="psum_misc", bufs=1, space="PSUM") as ps_misc,
        ):
            # ---- constants ----
            ident = constp.tile([P, P], f32)
            masks.make_identity(nc, ident[:])
            ones = constp.tile([P, 1], f32)
            nc.vector.memset(ones[:], 1.0)

            # ---- masks: load int32, cast to f32, transpose to [P, n_ex] cols ----
            qm_i = maskp.tile([n_ex, LQ], i32)
            nc.sync.dma_start(qm_i[:], qm_d[:, :])
            pm_i = maskp.tile([n_ex, LK], i32)
            nc.sync.dma_start(pm_i[:], pm_d[:, :])
            nm_i = maskp.tile([n_ex, LK], i32)
            nc.sync.dma_start(nm_i[:], nm_d[:, :])
            qm_f = maskp.tile([n_ex, LQ], f32)
            nc.vector.tensor_copy(qm_f[:], qm_i[:])
            pm_f = maskp.tile([n_ex, LK], f32)
            nc.vector.tensor_copy(pm_f[:], pm_i[:])
            nm_f = maskp.tile([n_ex, LK], f32)
            nc.vector.tensor_copy(nm_f[:], nm_i[:])

            # qmT: col = t*n_ex + ex ; kmT: col = (key*KT_TILES + t)*n_ex + ex
            qmT = constp.tile([P, QT_TILES * n_ex], f32)
            kmT = constp.tile([P, 2 * KT_TILES * n_ex], f32)
            for t in range(QT_TILES):
                pst = ps_misc.tile([P, n_ex], f32, tag="pmask")
                nc.tensor.transpose(
                    pst[:], qm_f[:, t * P:(t + 1) * P], ident[0:n_ex, 0:n_ex]
                )
                nc.scalar.copy(qmT[:, t * n_ex:(t + 1) * n_ex], pst[:])
            for key, m_f in ((0, pm_f), (1, nm_f)):
                for t in range(KT_TILES):
                    pst = ps_misc.tile([P, n_ex], f32, tag="pmask")
                    nc.tensor.transpose(
                        pst[:], m_f[:, t * P:(t + 1) * P], ident[0:n_ex, 0:n_ex]
                    )
                    c0 = (key * KT_TILES + t) * n_ex
                    nc.scalar.copy(kmT[:, c0:c0 + n_ex], pst[:])

            # ---- per-example state ----
            sumsq_q = constp.tile([P, QT_TILES * n_ex], f32)   # col = ex*2+t
            invq = constp.tile([P, QT_TILES * n_ex], f32)
            sumsq_k = constp.tile([P, 2 * KT_TILES * n_ex], f32)  # ex*8+key*4+t
            invk = constp.tile([P, 2 * KT_TILES * n_ex], f32)
            Z = constp.tile([P, 4 * n_ex], f32)  # col = ex*4 + key*2 + t

            for ex in range(n_ex):
                # -- queries: load natural, sumsq, transpose (un-normalized) --
                qnat = qnatp.tile([P, QT_TILES * D], f32, tag="qnat")
                for t in range(QT_TILES):
                    nc.sync.dma_start(
                        qnat[:, t * D:(t + 1) * D], q_d[ex, t * P:(t + 1) * P, :]
                    )
                for t in range(QT_TILES):
                    scr = scrp.tile([P, D], f32, tag="scr")
                    nc.scalar.activation(
                        scr[:], qnat[:, t * D:(t + 1) * D], Sq,
                        accum_out=sumsq_q[:, ex * QT_TILES + t: ex * QT_TILES + t + 1],
                    )
                nrmq = tmpp.tile([P, QT_TILES], f32, tag="nrmq")
                nc.scalar.sqrt(nrmq[:], sumsq_q[:, ex * QT_TILES:(ex + 1) * QT_TILES])
                nc.vector.reciprocal(
                    invq[:, ex * QT_TILES:(ex + 1) * QT_TILES], nrmq[:]
                )
                # qT layout: [p, d*LQ + t*P + i] = q[t*P+i, d*P+p]
                qT = qTp.tile([P, DC * LQ], mm_dt, tag="qT")
                for d in range(DC):
                    psq = ps_qT.tile([P, LQ], f32, tag="psq")
                    for t in range(QT_TILES):
                        nc.tensor.transpose(
                            psq[:, t * P:(t + 1) * P],
                            qnat[:, t * D + d * P: t * D + d * P + P],
                            ident[:],
                        )
                    nc.scalar.copy(qT[:, d * LQ:(d + 1) * LQ], psq[:])

                # -- keys --
                for key, k_d in ((0, pk_d), (1, nk_d)):
                    knat = knatp.tile([P, KT_TILES * D], f32, tag="knat")
                    for t in range(KT_TILES):
                        nc.sync.dma_start(
                            knat[:, t * D:(t + 1) * D], k_d[ex, t * P:(t + 1) * P, :]
                        )
                    kb = (ex * 2 + key) * KT_TILES  # sumsq_k/invk col base
                    for t in range(KT_TILES):
                        scr = scrp.tile([P, D], f32, tag="scr")
                        nc.scalar.activation(
                            scr[:], knat[:, t * D:(t + 1) * D], Sq,
                            accum_out=sumsq_k[:, kb + t: kb + t + 1],
                        )
                    nrmk = tmpp.tile([P, KT_TILES], f32, tag="nrmk")
                    nc.scalar.sqrt(nrmk[:], sumsq_k[:, kb: kb + KT_TILES])
                    nc.vector.reciprocal(invk[:, kb: kb + KT_TILES], nrmk[:])
                    # scale rows by mask/||k||  (both per-partition scalars)
                    for t in range(KT_TILES):
                        mc = (key * KT_TILES + t) * n_ex + ex
                        nc.vector.tensor_scalar(
                            knat[:, t * D:(t + 1) * D],
                            knat[:, t * D:(t + 1) * D],
                            invk[:, kb + t: kb + t + 1],
                            kmT[:, mc: mc + 1],
                            MULT, MULT,
                        )
                    # transpose to kT layout [p, d*LK + t*P + i] = kn[t*P+i, d*P+p]
                    kT = kTp.tile([P, DC * LK], mm_dt, tag="kT")
                    for d in range(DC):
                        psk = ps_kT.tile([P, LK], f32, tag="psk")
                        for t in range(KT_TILES):
                            nc.tensor.transpose(
                                psk[:, t * P:(t + 1) * P],
                                knat[:, t * D + d * P: t * D + d * P + P],
                                ident[:],
                            )
                        nc.vector.tensor_copy(kT[:, d * LK:(d + 1) * LK], psk[:])
                    # S = qT.T @ kT per query tile, accumulate over d
                    for t in range(QT_TILES):
                        S = ps_S.tile([P, LK], f32, tag="S")
                        for d in range(DC):
                            nc.tensor.matmul(
                                S[:],
                                qT[:, d * LQ + t * P: d * LQ + t * P + P],
                                kT[:, d * LK:(d + 1) * LK],
                                start=(d == 0),
                                stop=(d == DC - 1),
                            )
                        M = tmpp.tile([P, 1], f32, tag="M")
                        nc.vector.reduce_max(M[:], S[:], axis=X)
                        zc = ex * 4 + key * 2 + t
                        nc.vector.tensor_scalar(
                            Z[:, zc: zc + 1], M[:],
                            invq[:, ex * QT_TILES + t: ex * QT_TILES + t + 1],
                            qmT[:, t * n_ex + ex: t * n_ex + ex + 1],
                            MULT, MULT,
                        )

            # ---- final partition reduction: out[j] = sum_p Z[p, j] ----
            pso = ps_misc.tile([4 * n_ex, 1], f32, tag="pout")
            nc.tensor.matmul(pso[:], Z[:, 0:4 * n_ex], ones[:], start=True, stop=True)
            out_sb = constp.tile([4 * n_ex, 1], f32)
            nc.scalar.copy(out_sb[:], pso[:])
            nc.sync.dma_start(out_d[:, :], out_sb[:])

    return nc


_NC_CACHE = {}


def _get_nc(n_ex=EX, use_f32r=False):
    key = (n_ex, use_f32r)
    if key not in _NC_CACHE:
        _NC_CACHE[key] = _build(n_ex, use_f32r)
    return _NC_CACHE[key]


def _device_logits(query, pos_key, neg_key, query_mask, pos_mask, neg_mask,
                   use_f32r=True, **run_kwargs):
    """Run the device kernel; returns (pos_logits[B], neg_logits[B], results)."""
    from concourse.bass_utils import run_bass_kernel_spmd

    _install_patches()
    nc = _get_nc(EX, use_f32r)
    in_maps = []
    for c in range(NCORES):
        sl = slice(c * EX, (c + 1) * EX)
        in_maps.append({
            "q": np.ascontiguousarray(query[sl]),
            "pk": np.ascontiguousarray(pos_key[sl]),
            "nk": np.ascontiguousarray(neg_key[sl]),
            "qm": np.ascontiguousarray(query_mask[sl]),
            "pm": np.ascontiguousarray(pos_mask[sl]),
            "nm": np.ascontiguousarray(neg_mask[sl]),
        })
    res = run_bass_kernel_spmd(nc, in_maps, core_ids=list(range(NCORES)),
                               **run_kwargs)
    pos = np.empty(B, np.float64)
    neg = np.empty(B, np.float64)
    for c in range(NCORES):
        o = res.results[c]["out"].reshape(EX, 4).astype(np.float64)
        pos[c * EX:(c + 1) * EX] = o[:, 0] + o[:, 1]
        neg[c * EX:(c + 1) * EX] = o[:, 2] + o[:, 3]
    return pos, neg, res


def kernel(query, pos_key, neg_key, query_mask, pos_mask, neg_mask):
    pos, neg, _ = _device_logits(query, pos_key, neg_key,
                                 query_mask, pos_mask, neg_mask)
    nll = np.logaddexp(0.0, (neg - pos) / TEMP)
    return np.array(nll.mean(), dtype=np.float32)
